# revision 8
# baseline (speedup 1.0000x reference)
"""AudioTransformer Trainium2 kernel.

Sharding: 8 cores = 2 batch groups x 4 sequence quarters.
- Encoder/decoder: sharded over (batch, time) with host-supplied halos.
- Transformer: each core owns a 128-row chunk of S=512; per-layer AllGather
  (groups of 4) replicates the hidden state; windowed attention reads a
  384-row ext window via dynamic-offset transposed DMA from a zero-padded
  DRAM buffer.
Compute: bf16 matmul inputs, fp32 PSUM accumulation, fp32 residual/LN stream.
Biases enter matmuls as an extra contraction row against a ones vector.
"""

import numpy as np
import ml_dtypes

import concourse.bacc as bacc
import concourse.mybir as mybir
import concourse.tile as tile
from concourse.bass import ds
from concourse.bass_utils import run_bass_kernel_spmd
from concourse.masks import make_identity

AL = mybir.AluOpType
AF = mybir.ActivationFunctionType
AX = mybir.AxisListType
BF = mybir.dt.bfloat16
F32 = mybir.dt.float32
bf16 = ml_dtypes.bfloat16

N_MELS = 128
ENC = [256, 512, 512]
E = 512
FF = 2048
LIMIT = 128
NLAYERS = 4
B = 2
T = 4096
S = 512          # sequence length entering the transformer
CH = 128         # per-core S chunk
WIN = 384        # ext window rows per core
XW = 1088        # encoder input window width (1024 + 2*32)
NEG = -1.0e9

TRACE = False
LAST_EXEC_NS = None

_CACHE = {}


# ---------------------------------------------------------------- host prep

def _pe_table():
    pos = np.arange(S, dtype=np.float32)[:, None]
    div = np.exp(np.arange(0, E, 2, dtype=np.float32) * (-(np.log(10000.0) / E)))
    ang = (pos * div.astype(np.float32)[None, :]).astype(np.float32)
    pe = np.stack([np.sin(ang), np.cos(ang)], axis=-1).reshape(S, E)
    return pe.astype(np.float32)


def _host_inputs(x, enc_ws, enc_bs, layers, dec_ws, dec_bs):
    """Build the 8 per-core input maps."""
    x = np.asarray(x, np.float32)
    f32 = lambda a: np.asarray(a, np.float32)
    tobf = lambda a: np.ascontiguousarray(np.asarray(a, np.float32)).astype(bf16)

    pe = _pe_table()
    shared = {}

    # encoder weights: conv w [O, I, K] -> per tap lhsT [I, O], taps stacked
    for i, (w, b) in enumerate(zip(enc_ws, enc_bs)):
        w = f32(w)
        O, I, K = w.shape
        lhsT = np.concatenate([w[:, :, t].T for t in range(K)], axis=0)  # [K*I, O]
        shared[f"ew{i}"] = tobf(lhsT)
        shared[f"eb{i}"] = np.ascontiguousarray(f32(b).reshape(O, 1))

    # transformer weights, bias folded in as an extra contraction row
    sc = 1.0 / np.sqrt(np.float32(E))
    for l, p in enumerate(layers):
        wq = np.concatenate([f32(p["Wq"]), f32(p["bq"])[None, :]], 0) * sc
        wk = np.concatenate([f32(p["Wk"]), f32(p["bk"])[None, :]], 0)
        wv = np.concatenate([f32(p["Wv"]), f32(p["bv"])[None, :]], 0)
        w1 = np.concatenate([f32(p["W1"]), f32(p["b1f"])[None, :]], 0)
        w2 = np.concatenate([f32(p["W2"]), f32(p["b2f"])[None, :]], 0)
        shared[f"wq{l}"] = tobf(wq)      # [513, 512]
        shared[f"wk{l}"] = tobf(wk)
        shared[f"wv{l}"] = tobf(wv)
        shared[f"w1f{l}"] = tobf(w1)     # [513, 2048]
        shared[f"w2f{l}"] = tobf(w2)     # [2049, 512]
        ln = np.concatenate([
            np.tile(f32(p["g1"])[None, :], (128, 1)),
            np.tile(f32(p["be1"])[None, :], (128, 1)),
            np.tile(f32(p["g2"])[None, :], (128, 1)),
            np.tile(f32(p["be2"])[None, :], (128, 1)),
        ], axis=1)                        # [128, 4*512]
        shared[f"ln{l}"] = np.ascontiguousarray(ln)

    # decoder weights: w [I, O, K] -> taps stacked [K*I, O]
    for i, (w, b) in enumerate(zip(dec_ws, dec_bs)):
        w = f32(w)
        I, O, K = w.shape
        lhsT = np.concatenate([w[:, :, t] for t in range(K)], axis=0)  # [K*I, O]
        shared[f"dw{i}"] = tobf(lhsT)
        shared[f"db{i}"] = np.ascontiguousarray(f32(b).reshape(O, 1))

    # validity masks for stage outputs at global sequence edges: stream
    # formulas produce phantom values at positions outside the true output
    # range; zero them before the next stage consumes them.
    def colmask(width, lo, length):
        v = ((np.arange(width) + lo >= 0) & (np.arange(width) + lo < length))
        return np.tile(v.astype(np.float32)[None, :], (128, 1)).astype(bf16)

    # band mask [128, 384]
    ii = np.arange(CH)[:, None]
    jj = np.arange(WIN)[None, :]
    mask = np.where((jj >= ii) & (jj <= ii + 2 * LIMIT), 0.0, NEG).astype(np.float32)
    shared["mask"] = mask

    in_maps = []
    for c in range(8):
        b_, r = c // 4, c % 4
        m = dict(shared)
        # encoder input window [128, 1088]: global t in [1024r-32, 1024r+1056)
        lo, hi = 1024 * r - 32, 1024 * r + 1056
        xw = np.zeros((N_MELS, XW), np.float32)
        glo, ghi = max(lo, 0), min(hi, T)
        xw[:, glo - lo:ghi - lo] = x[b_, :, glo:ghi]
        m["x_win"] = xw.astype(bf16)
        # pe for own chunk [128, 512] f32
        m["pe_own"] = np.ascontiguousarray(pe[CH * r:CH * (r + 1)])
        # pe for the ext window, transposed [512, 384] bf16, zero at pads
        pw = np.zeros((WIN, E), np.float32)
        slo, shi = max(CH * r - 128, 0), min(CH * r + 256, S)
        pw[slo - (CH * r - 128):shi - (CH * r - 128)] = pe[slo:shi]
        m["pe_winT"] = np.ascontiguousarray(pw.T).astype(bf16)
        m["e1m"] = colmask(543, 512 * r - 16, 2048)
        m["e2m"] = colmask(270, 256 * r - 8, 1024)
        m["d1m"] = colmask(320, 256 * r - 32, 1024)
        m["d2m"] = colmask(640, 512 * r - 64, 2048)
        in_maps.append(m)
    return in_maps


# ---------------------------------------------------------------- device code

def _declare_inputs(nc):
    d = {}

    def inp(name, shape, dt):
        d[name] = nc.dram_tensor(name, list(shape), dt, kind="ExternalInput")

    inp("x_win", (N_MELS, XW), BF)
    cin = N_MELS
    for i, cout in enumerate(ENC):
        inp(f"ew{i}", (5 * cin, cout), BF)
        inp(f"eb{i}", (cout, 1), F32)
        cin = cout
    for l in range(NLAYERS):
        inp(f"wq{l}", (E + 1, E), BF)
        inp(f"wk{l}", (E + 1, E), BF)
        inp(f"wv{l}", (E + 1, E), BF)
        inp(f"w1f{l}", (E + 1, FF), BF)
        inp(f"w2f{l}", (FF + 1, E), BF)
        inp(f"ln{l}", (128, 4 * E), F32)
    for i in range(3):
        O = 1 if i == 2 else E
        inp(f"dw{i}", (4 * E, O), BF)
        inp(f"db{i}", (O, 1), F32)
    inp("pe_own", (CH, E), F32)
    inp("pe_winT", (E, WIN), BF)
    inp("mask", (CH, WIN), F32)
    inp("e1m", (128, 543), BF)
    inp("e2m", (128, 270), BF)
    inp("d1m", (128, 320), BF)
    inp("d2m", (128, 640), BF)
    d["out"] = nc.dram_tensor("out", [1, 1024], F32, kind="ExternalOutput")
    return d


def _encoder(nc, tc, io, ps, h0_own):
    """conv/pool stack -> h0_own [128 s, 512 e] f32 (own chunk)."""
    with tc.tile_pool(name="enc", bufs=1) as enc:
        w_sb = []
        cin = N_MELS
        for i, cout in enumerate(ENC):
            kt = cin // 128
            wt = enc.tile([128, 5 * kt * cout], BF, name=f"ewsb{i}")
            nc.sync.dma_start(
                out=wt[:], in_=io[f"ew{i}"].rearrange("(tk p) o -> p tk o", p=128))
            bt = enc.tile([128, cout // 128], F32, name=f"ebsb{i}")
            nc.sync.dma_start(
                out=bt[:], in_=io[f"eb{i}"].rearrange("(m p) one -> p (m one)", p=128))
            w_sb.append((wt, bt, kt, cout))
            cin = cout

        xb = enc.tile([128, XW], BF)
        nc.sync.dma_start(out=xb[:], in_=io["x_win"][:])

        def conv_pool(inp_tiles, stage, out_cols, chunks, out_f32=False):
            wt, bt, kt, cout = w_sb[stage]
            mt = cout // 128
            outs = [
                enc.tile([128, out_cols], F32 if out_f32 else BF, name=f"h{stage}_{m}")
                for m in range(mt)
            ]
            for m in range(mt):
                for (t0, Wc, q0) in chunks:
                    pst = ps.tile([128, 512], F32, name=f"pse{stage}_{m}_{q0}",
                                  tag="mm", bufs=3)
                    first = True
                    for k in range(kt):
                        for tap in range(5):
                            col = (tap * kt + k) * cout + m * 128
                            nc.tensor.matmul(
                                pst[:, :Wc],
                                wt[:, col:col + 128],
                                inp_tiles[k][:, t0 - 2 + tap:t0 - 2 + tap + Wc],
                                start=first, stop=(k == kt - 1 and tap == 4),
                            )
                            first = False
                    half = Wc // 2
                    tmp = enc.tile([128, 512], F32, name="pooltmp",
                                   tag="pooltmp", bufs=2)
                    nc.vector.tensor_scalar(
                        out=tmp[:, :Wc], in0=pst[:, :Wc],
                        scalar1=bt[:, m:m + 1], scalar2=0.0, op0=AL.add, op1=AL.max)
                    t3 = tmp[:, :Wc].rearrange("p (t two) -> p t two", two=2)
                    nc.vector.tensor_tensor(
                        out=outs[m][:, q0:q0 + half], in0=t3[:, :, 0],
                        in1=t3[:, :, 1], op=AL.max)
            return outs

        e1m = enc.tile([128, 543], BF, name="e1m_sb")
        nc.sync.dma_start(out=e1m[:], in_=io["e1m"][:])
        e2m = enc.tile([128, 270], BF, name="e2m_sb")
        nc.sync.dma_start(out=e2m[:], in_=io["e2m"][:])

        h1 = conv_pool([xb], 0, 543, [(2, 512, 1), (514, 512, 257), (1026, 60, 513)])
        for t_ in h1:
            nc.vector.tensor_tensor(out=t_[:], in0=t_[:], in1=e1m[:], op=AL.mult)
        h2 = conv_pool(h1, 1, 270, [(4, 512, 2), (516, 24, 258)])
        for t_ in h2:
            nc.vector.tensor_tensor(out=t_[:], in0=t_[:], in1=e2m[:], op=AL.mult)
        h3 = conv_pool(h2, 2, 133, [(4, 262, 2)], out_f32=True)

        ident = enc.tile([128, 128], F32, name="identf32_enc")
        make_identity(nc, ident[:])
        for et in range(4):
            pst = ps.tile([128, 128], F32, name=f"ptr_enc{et}", tag="tr", bufs=2)
            nc.tensor.transpose(pst[:], h3[et][:, 4:132], ident[:])
            nc.vector.tensor_copy(h0_own[:, et * 128:(et + 1) * 128], pst[:])


def _layernorm(nc, pool, x_ap, g_ap, b_ap, out_ap, eps_ap):
    st = lambda shp, name: pool.tile(shp, F32, name=name, tag=f"ln_{name}", bufs=1)
    srow = st([128, 1], "srow")
    nc.vector.reduce_sum(out=srow[:], in_=x_ap, axis=AX.X)
    mean = st([128, 1], "mean")
    nc.vector.tensor_scalar_mul(mean[:], srow[:], 1.0 / E)
    xm = st([128, E], "xm")
    nc.vector.tensor_scalar_sub(xm[:], x_ap, mean[:])
    sq = st([128, E], "sq")
    nc.vector.tensor_tensor(out=sq[:], in0=xm[:], in1=xm[:], op=AL.mult)
    ssq = st([128, 1], "ssq")
    nc.vector.reduce_sum(out=ssq[:], in_=sq[:], axis=AX.X)
    std = st([128, 1], "std")
    nc.scalar.activation(std[:], ssq[:], AF.Sqrt, bias=eps_ap, scale=1.0 / E)
    rstd = st([128, 1], "rstd")
    nc.vector.reciprocal(rstd[:], std[:])
    xn = st([128, E], "xn")
    nc.vector.tensor_scalar_mul(xn[:], xm[:], rstd[:])
    tmp = st([128, E], "gmul")
    nc.vector.tensor_tensor(out=tmp[:], in0=xn[:], in1=g_ap, op=AL.mult)
    nc.vector.tensor_tensor(out=out_ap, in0=tmp[:], in1=b_ap, op=AL.add)


def _build_nc():
    nc = bacc.Bacc("TRN2", target_bir_lowering=False, debug=False, num_devices=8)
    io = _declare_inputs(nc)
    groups = [[0, 1, 2, 3], [4, 5, 6, 7]]

    with tile.TileContext(nc) as tc:
        with (
            tc.tile_pool(name="const", bufs=1) as const,
            tc.tile_pool(name="wl", bufs=2) as wl,
            tc.tile_pool(name="act", bufs=2) as act,
            tc.tile_pool(name="hst", bufs=2) as hst,
            tc.tile_pool(name="ps", bufs=1, space="PSUM") as ps,
            tc.tile_pool(name="dram", bufs=1, space="DRAM") as dram,
        ):
            # ---------------- constants
            ident_bf = const.tile([128, 128], BF)
            make_identity(nc, ident_bf[:])
            ones = const.tile([1, E], BF)
            nc.vector.memset(ones[:], 1.0)
            mask_sb = const.tile([CH, WIN], F32)
            nc.sync.dma_start(out=mask_sb[:], in_=io["mask"][:])
            pe_own = const.tile([CH, E], F32)
            nc.sync.dma_start(out=pe_own[:], in_=io["pe_own"][:])
            peT = const.tile([128, 4 * WIN], BF)
            nc.sync.dma_start(
                out=peT[:], in_=io["pe_winT"].rearrange("(et p) w -> p et w", p=128))

            eps_t = const.tile([128, 1], F32)
            nc.vector.memset(eps_t[:], 1e-5)
            zt = const.tile([128, E], BF)
            nc.vector.memset(zt[:], 0.0)
            PA = dram.tile([768, E], BF, name="PA")
            PB = dram.tile([768, E], BF, name="PB")
            for P_ in (PA, PB):
                nc.sync.dma_start(out=P_[0:128, :], in_=zt[:])
                nc.sync.dma_start(out=P_[640:768, :], in_=zt[:])

            pid = nc.sync.partition_id()
            rk = pid % 4
            off_win = rk * 128          # P rows [128r, 128r+384)
            off_dec = rk * 128 + 112    # P rows for the decoder window (160)

            # ---------------- encoder
            h_own = hst.tile([CH, E], F32, name="h_own0", tag="h_own")
            _encoder(nc, tc, io, ps, h_own)

            def send_h(h_f32, P_out, lname):
                hb = act.tile([CH, E], BF, name=f"hbf_{lname}", tag="hbf")
                nc.vector.tensor_copy(hb[:], h_f32[:])
                bounce = dram.tile([CH, E], BF, name=f"bounce_{lname}",
                                   tag="bounce", bufs=2)
                nc.sync.dma_start(out=bounce[:], in_=hb[:])
                nc.gpsimd.collective_compute(
                    "AllGather", AL.bypass,
                    ins=[bounce[:].opt()],
                    outs=[P_out[128:640, :].opt()],
                    replica_groups=groups,
                )

            send_h(h_own, PA, "h0")

            # ---------------- transformer layers
            P_in, P_out = PA, PB
            for l in range(NLAYERS):
                def wload(name, rows, cols, tag, nb=2):
                    kt = rows // 128
                    t_ = wl.tile([128, kt * cols], BF, name=f"{tag}_{l}", tag=tag,
                                 bufs=nb)
                    nc.sync.dma_start(
                        out=t_[:],
                        in_=io[name][:kt * 128].rearrange("(k p) c -> p k c", p=128))
                    rowt = wl.tile([1, cols], BF, name=f"{tag}r_{l}", tag=f"{tag}r")
                    nc.sync.dma_start(out=rowt[:],
                                      in_=io[name][kt * 128:kt * 128 + 1, :])
                    return t_, rowt

                wq, wqr = wload(f"wq{l}", E, E, "wq")
                wk, wkr = wload(f"wk{l}", E, E, "wk")
                wv, wvr = wload(f"wv{l}", E, E, "wv")
                w1, w1r = wload(f"w1f{l}", E, FF, "w1f", nb=1)
                w2, w2r = wload(f"w2f{l}", FF, E, "w2f", nb=1)
                lnt = wl.tile([128, 4 * E], F32, name=f"lnsb_{l}", tag="lnsb",
                              bufs=1)
                nc.sync.dma_start(out=lnt[:], in_=io[f"ln{l}"][:])
                g1, be1 = lnt[:, 0:E], lnt[:, E:2 * E]
                g2, be2 = lnt[:, 2 * E:3 * E], lnt[:, 3 * E:4 * E]

                # window (transposed) + pe -> hp^T [e, t] bf16, 4 tiles [128, 384]
                hpT = []
                for et in range(4):
                    wt_ = act.tile([128, WIN], BF, name=f"hpT{et}_{l}", tag=f"hpT{et}")
                    nc.sync.dma_start(
                        out=wt_[:],
                        in_=P_in[ds(off_win, WIN), et * 128:(et + 1) * 128],
                        transpose=True)
                    nc.vector.tensor_tensor(
                        out=wt_[:], in0=wt_[:],
                        in1=peT[:, et * WIN:(et + 1) * WIN], op=AL.add)
                    hpT.append(wt_)

                hp_own = act.tile([CH, E], F32, name=f"hp_own_{l}", tag="hp_own")
                nc.vector.tensor_tensor(out=hp_own[:], in0=h_own[:], in1=pe_own[:],
                                        op=AL.add)

                # q^T [e', s]: 4 accumulation groups -> packed SBUF [128, 512]
                qT = act.tile([128, 512], BF, name=f"qT_{l}", tag="qT")
                for m in range(4):
                    ps_q = ps.tile([128, 128], F32, name=f"psq{m}_{l}",
                                   tag="mm128", bufs=3)
                    for k in range(4):
                        nc.tensor.matmul(
                            ps_q[:],
                            wq[:, k * E + m * 128:k * E + (m + 1) * 128],
                            hpT[k][:, 128:256],
                            start=(k == 0), stop=False)
                    nc.tensor.matmul(
                        ps_q[:], wqr[:, m * 128:(m + 1) * 128], ones[:, 0:128],
                        start=False, stop=True)
                    nc.vector.tensor_copy(qT[:, m * 128:(m + 1) * 128], ps_q[:])

                # k^T [e', t]: 4 tiles [128, 384]
                kT = []
                for m in range(4):
                    ps_k = ps.tile([128, 512], F32, name=f"psk{m}_{l}",
                                   tag="mm", bufs=3)
                    for k in range(4):
                        nc.tensor.matmul(
                            ps_k[:, :WIN],
                            wk[:, k * E + m * 128:k * E + (m + 1) * 128],
                            hpT[k][:],
                            start=(k == 0), stop=False)
                    nc.tensor.matmul(
                        ps_k[:, :WIN], wkr[:, m * 128:(m + 1) * 128], ones[:, 0:WIN],
                        start=False, stop=True)
                    kt_ = act.tile([128, WIN], BF, name=f"kT{m}_{l}", tag=f"kT{m}")
                    nc.vector.tensor_copy(kt_[:], ps_k[:, :WIN])
                    kT.append(kt_)

                # v [t, e']: 3 tiles [128, 512]
                vsb = []
                for mt in range(3):
                    ps_v = ps.tile([128, 512], F32, name=f"psv{mt}_{l}",
                                   tag="mm", bufs=3)
                    for k in range(4):
                        nc.tensor.matmul(
                            ps_v[:],
                            hpT[k][:, mt * 128:(mt + 1) * 128],
                            wv[:, k * E:(k + 1) * E],
                            start=(k == 0), stop=False)
                    nc.tensor.matmul(ps_v[:], ones[:, 0:128], wvr[:],
                                     start=False, stop=True)
                    vt_ = act.tile([128, E], BF, name=f"v{mt}_{l}", tag=f"v{mt}")
                    nc.vector.tensor_copy(vt_[:], ps_v[:])
                    vsb.append(vt_)

                # energy + softmax
                ps_e = ps.tile([128, 512], F32, name=f"pse_{l}", tag="mm", bufs=3)
                for k in range(4):
                    nc.tensor.matmul(
                        ps_e[:, :WIN], qT[:, k * 128:(k + 1) * 128], kT[k][:],
                        start=(k == 0), stop=(k == 3))
                ef = act.tile([CH, WIN], F32, name=f"ef_{l}", tag="ef")
                nc.vector.tensor_tensor(out=ef[:], in0=ps_e[:, :WIN],
                                        in1=mask_sb[:], op=AL.add)
                mx = act.tile([CH, 1], F32, name=f"mx_{l}", tag="mx")
                nc.vector.reduce_max(out=mx[:], in_=ef[:], axis=AX.X)
                nmx = act.tile([CH, 1], F32, name=f"nmx_{l}", tag="nmx")
                nc.vector.tensor_scalar_mul(nmx[:], mx[:], -1.0)
                pf = act.tile([CH, WIN], F32, name=f"pf_{l}", tag="pf")
                nc.scalar.activation(pf[:], ef[:], AF.Exp, bias=nmx[:], scale=1.0)
                sm = act.tile([CH, 1], F32, name=f"sm_{l}", tag="sm")
                nc.vector.reduce_sum(out=sm[:], in_=pf[:], axis=AX.X)
                rs = act.tile([CH, 1], F32, name=f"rs_{l}", tag="rs")
                nc.vector.reciprocal(rs[:], sm[:])
                attn = act.tile([CH, WIN], BF, name=f"attn_{l}", tag="attn")
                nc.vector.tensor_scalar_mul(attn[:], pf[:], rs[:])

                # attn^T (3 PE transposes) then a = attn @ v  [s, e]
                aT = []
                for j in range(3):
                    ps_t = ps.tile([128, 128], BF, name=f"pstr{j}_{l}",
                                   tag="tr", bufs=2)
                    nc.tensor.transpose(ps_t[:], attn[:, j * 128:(j + 1) * 128],
                                        ident_bf[:])
                    at_ = act.tile([128, 128], BF, name=f"aT{j}_{l}", tag=f"aT{j}")
                    nc.vector.tensor_copy(at_[:], ps_t[:])
                    aT.append(at_)
                ps_a = ps.tile([128, 512], F32, name=f"psa_{l}", tag="mm", bufs=3)
                for j in range(3):
                    nc.tensor.matmul(ps_a[:], aT[j][:], vsb[j][:],
                                     start=(j == 0), stop=(j == 2))

                r1 = act.tile([CH, E], F32, name=f"r1_{l}", tag="r1")
                nc.vector.tensor_tensor(out=r1[:], in0=ps_a[:], in1=hp_own[:],
                                        op=AL.add)
                h1 = hst.tile([CH, E], F32, name=f"h1_{l}", tag="h1")
                _layernorm(nc, act, r1[:], g1, be1, h1[:], eps_t[:])

                # h1^T bf16
                h1b = act.tile([CH, E], BF, name=f"h1b_{l}", tag="h1b")
                nc.vector.tensor_copy(h1b[:], h1[:])
                h1T = []
                for et in range(4):
                    ps_t = ps.tile([128, 128], BF, name=f"psh{et}_{l}",
                                   tag="tr", bufs=2)
                    nc.tensor.transpose(ps_t[:], h1b[:, et * 128:(et + 1) * 128],
                                        ident_bf[:])
                    ht_ = act.tile([128, 128], BF, name=f"h1T{et}_{l}", tag=f"h1T{et}")
                    nc.vector.tensor_copy(ht_[:], ps_t[:])
                    h1T.append(ht_)

                # FFN: g^T [ff, s] 16 groups, relu; then y [s, e]
                gT = []
                for b_ in range(4):
                    gt_ = act.tile([128, 512], BF, name=f"gT{b_}_{l}", tag=f"gT{b_}")
                    for j in range(4):
                        mm = 4 * b_ + j
                        ps_g = ps.tile([128, 128], F32, name=f"psg{mm}_{l}",
                                       tag="mm128", bufs=3)
                        for k in range(4):
                            nc.tensor.matmul(
                                ps_g[:],
                                w1[:, k * FF + mm * 128:k * FF + (mm + 1) * 128],
                                h1T[k][:],
                                start=(k == 0), stop=False)
                        nc.tensor.matmul(
                            ps_g[:], w1r[:, mm * 128:(mm + 1) * 128], ones[:, 0:128],
                            start=False, stop=True)
                        nc.vector.tensor_scalar_max(
                            gt_[:, j * 128:(j + 1) * 128], ps_g[:], 0.0)
                    gT.append(gt_)

                ps_y = ps.tile([128, 512], F32, name=f"psy_{l}", tag="mm", bufs=3)
                for k in range(16):
                    nc.tensor.matmul(
                        ps_y[:],
                        gT[k // 4][:, (k % 4) * 128:(k % 4 + 1) * 128],
                        w2[:, k * E:(k + 1) * E],
                        start=(k == 0), stop=False)
                nc.tensor.matmul(ps_y[:], ones[:, 0:128], w2r[:],
                                 start=False, stop=True)

                r2 = act.tile([CH, E], F32, name=f"r2_{l}", tag="r2")
                nc.vector.tensor_tensor(out=r2[:], in0=ps_y[:], in1=h1[:], op=AL.add)
                h_own = hst.tile([CH, E], F32, name=f"h_own{l + 1}", tag="h_own")
                _layernorm(nc, act, r2[:], g2, be2, h_own[:], eps_t[:])

                send_h(h_own, P_out, f"h{l + 1}")
                P_in, P_out = P_out, P_in

            # ---------------- decoder (reads P_in = last gathered h)
            with tc.tile_pool(name="dec", bufs=1) as dec:
                dw = []
                for i in range(3):
                    O = 1 if i == 2 else E
                    if i < 2:
                        t_ = dec.tile([128, 16 * O], BF, name=f"dwsb{i}")
                        nc.sync.dma_start(
                            out=t_[:],
                            in_=io[f"dw{i}"].rearrange("(tk p) o -> p tk o", p=128))
                        bt_ = dec.tile([128, 4], F32, name=f"dbsb{i}")
                        nc.sync.dma_start(
                            out=bt_[:],
                            in_=io[f"db{i}"].rearrange("(m p) one -> p (m one)", p=128))
                    else:
                        t_ = dec.tile([128, 16], BF, name=f"dwsb{i}")
                        nc.sync.dma_start(
                            out=t_[:],
                            in_=io[f"dw{i}"].rearrange("(tk p) one -> p (tk one)", p=128))
                        bt_ = dec.tile([1, 1], F32, name=f"dbsb{i}")
                        nc.sync.dma_start(out=bt_[:], in_=io[f"db{i}"][:])
                    dw.append((t_, bt_))

                # h4^T window [4 x [128, 160]]
                h4T = []
                for et in range(4):
                    wt_ = dec.tile([128, 160], BF, name=f"h4T{et}")
                    nc.sync.dma_start(
                        out=wt_[:],
                        in_=P_in[ds(off_dec, 160), et * 128:(et + 1) * 128],
                        transpose=True)
                    h4T.append(wt_)

                def dec_stage(i, inp_tiles, ev_rng, od_rng, out_w, relu):
                    """ConvTranspose stage i.
                    even stream: out[c] = w3*in[c-1] + w1*in[c],  c in ev_rng
                    odd  stream: out[c] = w2*in[c]   + w0*in[c+1], c in od_rng
                    even c -> out col 2c (+offset), odd c -> 2c+1."""
                    wt_, bt_ = dw[i]
                    O = 1 if i == 2 else E
                    mt = 1 if i == 2 else 4
                    if i < 2:
                        outs = [dec.tile([128, out_w], BF, name=f"d{i}i{m}")
                                for m in range(mt)]
                        for m in range(mt):
                            nc.vector.memset(outs[m][:], 0.0)
                    else:
                        outs = [dec.tile([1, 1024], F32, name="final")]
                    for m in range(mt):
                        for (c0, c1), taps, par, base in (
                            (ev_rng, (3, 1), 0, ev_rng[0] - 1),
                            (od_rng, (2, 0), 1, od_rng[0]),
                        ):
                            Wc = c1 - c0
                            pp = 128 if i < 2 else 1
                            ps_d = ps.tile([pp, 512], F32, name=f"psd{i}_{m}_{par}",
                                           tag="mm", bufs=3)
                            first = True
                            for tshift, tap in ((0, taps[0]), (1, taps[1])):
                                for k in range(4):
                                    if i < 2:
                                        lhs = wt_[:, (tap * 4 + k) * O + m * 128:
                                                  (tap * 4 + k) * O + (m + 1) * 128]
                                    else:
                                        lhs = wt_[:, tap * 4 + k:tap * 4 + k + 1]
                                    nc.tensor.matmul(
                                        ps_d[:pp, :Wc],
                                        lhs,
                                        inp_tiles[k][:, base + tshift:
                                                     base + tshift + Wc],
                                        start=first,
                                        stop=(tshift == 1 and k == 3))
                                    first = False
                            if i < 2:
                                o3 = outs[m].rearrange("p (t two) -> p t two", two=2)
                                dst = o3[:, c0:c1, par]
                                if relu:
                                    nc.vector.tensor_scalar(
                                        out=dst, in0=ps_d[:, :Wc],
                                        scalar1=bt_[:, m:m + 1], scalar2=0.0,
                                        op0=AL.add, op1=AL.max)
                                else:
                                    nc.vector.tensor_scalar_add(
                                        dst, ps_d[:, :Wc], bt_[:, m:m + 1])
                            else:
                                o3 = outs[0].rearrange("p (t two) -> p t two", two=2)
                                dst = o3[:, c0 - 64:c1 - 64, par]
                                nc.vector.tensor_scalar_add(
                                    dst, ps_d[:1, :Wc], bt_[0:1, 0:1])
                    return outs

                d1m = dec.tile([128, 320], BF, name="d1m_sb")
                nc.sync.dma_start(out=d1m[:], in_=io["d1m"][:])
                d2m = dec.tile([128, 640], BF, name="d2m_sb")
                nc.sync.dma_start(out=d2m[:], in_=io["d2m"][:])

                d1 = dec_stage(0, h4T, (1, 160), (0, 159), 320, relu=True)
                for t_ in d1:
                    nc.vector.tensor_tensor(out=t_[:], in0=t_[:], in1=d1m[:], op=AL.mult)
                d2 = dec_stage(1, d1, (2, 320), (1, 319), 640, relu=True)
                for t_ in d2:
                    nc.vector.tensor_tensor(out=t_[:], in0=t_[:], in1=d2m[:], op=AL.mult)
                fin = dec_stage(2, d2, (64, 576), (64, 576), 1024, relu=False)

                nc.sync.dma_start(out=io["out"][:], in_=fin[0][:])

    nc.finalize()
    return nc


def _get_nc():
    if "nc" not in _CACHE:
        _CACHE["nc"] = _build_nc()
    return _CACHE["nc"]


def _maybe_register_trace_hook():
    try:
        import sys, types
        if "antenv.axon_hooks" not in sys.modules:
            mod = types.ModuleType("antenv.axon_hooks")
            mod._hook = None
            mod.set_axon_ntff_profile_hook = lambda h: setattr(mod, "_hook", h)
            mod.get_axon_ntff_profile_hook = lambda: mod._hook
            sys.modules["antenv.axon_hooks"] = mod
            import antenv
            antenv.axon_hooks = mod
            from trn_agent_boot.trn_boot import _ntff_profile_via_ctypes
            mod.set_axon_ntff_profile_hook(
                _ntff_profile_via_ctypes("/opt/axon/libaxon_pjrt.so"))
    except Exception:
        pass


def kernel(x, enc_ws, enc_bs, layers, dec_ws, dec_bs):
    global LAST_EXEC_NS
    nc = _get_nc()
    in_maps = _host_inputs(x, enc_ws, enc_bs, layers, dec_ws, dec_bs)
    kw = {}
    if TRACE:
        _maybe_register_trace_hook()
        kw["trace"] = True
    res = run_bass_kernel_spmd(nc, in_maps, core_ids=list(range(8)), **kw)
    LAST_EXEC_NS = res.exec_time_ns
    out = np.zeros((B, 1, T), np.float32)
    for c in range(8):
        b_, r = c // 4, c % 4
        out[b_, 0, 1024 * r:1024 * (r + 1)] = res.results[c]["out"][0]
    return out
